# revision 5
# baseline (speedup 1.0000x reference)
"""AverageSpanExtractor Trainium2 kernel.

Math: out[b, n, :] = mean(seq[b, start_n:end_n, :]) * mask[b, n]

Strategy (per core; data-parallel over batch across 8 cores):
  1. Load seq [S=2048, D=512] f32, cast fp16.
  2. Per 128-token block: in-block inclusive cumsum via PE matmul with an
     upper-triangular ones matrix; cast PSUM to fp16 (ACT/DVE alternating)
     and store the UNOFFSET cumsums to a DRAM table [2048, 512] fp16
     (token i = in-block sum ending at i; no block offsets on this path).
  3. Gather token end-1 and token max(start-1, 0) for all spans with
     gpsimd.dma_gather on 4 parallel SWDGE queues (fp16 rows, 1KiB/desc).
  4. Post-gather correction: the missing block offsets (and the start==0
     edge case) are a tiny matmul C_j = A_j.T @ T17, where A_j [17, 128]
     is a host-computed selector (block-offset indicator * span scale) and
     T17 holds the 16 block totals (table row 127 of each block) plus
     token 0 (= seq row 0).  out_j = (G_end - G_start)*scale + C_j with the
     sub on GpSimd, scale-mul on ACT, add on DVE (parallel tail engines).
  5. All index-derived tensors (gather idx list, per-span scale, selectors)
     and the triangular constant come precomputed from the host.
"""

import numpy as np

import concourse.bacc as bacc
import concourse.bass as bass
import concourse.tile as tile
from concourse import mybir
from concourse.bass import AP
from concourse.library_config import mlp
from concourse.tile_rust import add_dep_helper

# Problem shape (hardcoded per contract).
B, S, D, N = 8, 2048, 512, 1024
NBLK = S // 128          # 16 token blocks
NTILE = N // 128         # 8 span tiles
NGATHER = 4              # gather instructions (2 span tiles each), 1 queue each
NQUAD = 4                # seq load / table store granularity: 4 blocks

F32 = mybir.dt.float32
I32 = mybir.dt.int32
I16 = mybir.dt.int16
F16 = mybir.dt.float16


def build_kernel_body(tc: tile.TileContext, seq: AP, idx16_in: AP, scale_in: AP,
                      asel_in: AP, utri_in: AP, out: AP, ctx):
    nc = tc.nc
    sbuf = ctx.enter_context(tc.tile_pool(name="sbuf", bufs=1))
    gpool = ctx.enter_context(tc.tile_pool(name="gpool", bufs=1))
    dpool = ctx.enter_context(tc.tile_pool(name="dpool", bufs=3))
    opool = ctx.enter_context(tc.tile_pool(name="opool", bufs=3))
    psum_e = ctx.enter_context(tc.tile_pool(name="pe", bufs=5, space="PSUM"))
    psum_c = ctx.enter_context(tc.tile_pool(name="pc", bufs=3, space="PSUM"))
    dram = ctx.enter_context(tc.tile_pool(name="dram", bufs=1, space="DRAM"))

    table = dram.tile([S, D], F16)

    # gather ucode load leads the GpSimd queue (~11us DMA trickle).
    nc.gpsimd.load_library(mlp)

    # ---------------- host-precomputed tensors (ACT queue) -----------------
    u_tri = sbuf.tile([128, 128], F16, tag="u_tri")
    nc.scalar.dma_start(u_tri[:], utri_in)
    idx16 = sbuf.tile([128, 128], I16, tag="idx16")
    nc.scalar.dma_start(idx16[:], idx16_in)
    scale = sbuf.tile([128, NTILE], F32, tag="scale")
    nc.scalar.dma_start(scale[:], scale_in)
    asel = sbuf.tile([17, N], F16, tag="asel")
    nc.scalar.dma_start(asel[:], asel_in)

    # ---------------- seq loads (Sync HWDGE), cast to fp16 on DVE ----------
    xbig = sbuf.tile([128, NBLK, D], F32, tag="xbig")
    xf = sbuf.tile([128, NBLK, D], F16, tag="xf")
    for q in range(NQUAD):
        sl = (slice(None), slice(4 * q, 4 * q + 4), slice(None))
        nc.sync.dma_start(
            xbig[sl],
            seq[512 * q:512 * (q + 1), :].rearrange("(j p) d -> p j d", p=128))
        nc.vector.tensor_copy(xf[sl], xbig[sl])

    # ------- prepare gathers early (idle Q7 cores), trigger later ----------
    # Traced BEFORE any table store so the preps carry no RAW dep on the
    # table; each trigger gets explicit deps on the stores instead.
    gsems = [ctx.enter_context(nc.semaphore(f"gsem{t}"))
             for t in range(NGATHER)]
    gts = []
    for t in range(NGATHER):
        g_t = gpool.tile([128, 4, D], F16, tag=f"g{t}")
        nc.gpsimd.dma_gather(
            out_ap=g_t[:], in_ap=table[:],
            idxs_ap=idx16[:, 32 * t:32 * t + 32],
            num_idxs=512, num_idxs_reg=512, elem_size=D,
            prepare_only=True, sem=gsems[t], queue_num=t)
        gts.append(g_t)

    # ---------------- in-block cumsums -> fp16 table stores ----------------
    # L_b = u_tri.T @ xf_b (inclusive cumsum); ACT/DVE alternate casting
    # PSUM f32 -> fp16 into ebig; one store DMA per quad of blocks,
    # alternating Sync/ACT HWDGE queues.
    ebig = sbuf.tile([128, NBLK, D], F16, tag="ebig")
    store_insts = []
    for q in range(NQUAD):
        for bb in range(4):
            b = 4 * q + bb
            pl = psum_e.tile([128, D], F32, tag="pe")
            nc.tensor.matmul(out=pl[:], lhsT=u_tri[:],
                             rhs=xf[:, b, :], start=True, stop=True)
            if b % 2 == 0:
                nc.scalar.copy(ebig[:, b, :], pl[:])
            else:
                nc.vector.tensor_copy(ebig[:, b, :], pl[:])
        eng = nc.sync if q % 2 == 0 else nc.scalar
        store_insts.append(eng.dma_start(
            table[512 * q:512 * (q + 1), :].rearrange("(j p) d -> p j d", p=128),
            ebig[:, 4 * q:4 * q + 4, :]))

    # T17: rows 0..15 = block totals (ebig partition 127), row 16 = token 0
    # (= seq row 0 = ebig[0, 0, :]) for the start==0 redirect.
    t17 = sbuf.tile([17, D], F16, tag="t17")
    nc.sync.dma_start(t17[0:NBLK, :], ebig[127:128, 0:NBLK, :])
    nc.sync.dma_start(t17[NBLK:NBLK + 1, :], ebig[0:1, 0, :])

    # ---------------- fire prepared gathers (4 parallel queues) ------------
    trigs = []
    for t in range(NGATHER):
        trig = nc.gpsimd.trigger_dma(count=None, queue_num=t)
        for st in store_insts:
            add_dep_helper(trig.ins, st.ins, sync=True,
                           reason="gather reads whole table")
        trigs.append(trig)

    # ---------------- combine: (G_end - G_start)*scale + A.T @ T17 ---------
    # sub on GpSimd, C-matmul on PE, scale-mul on ACT, add on DVE, store on
    # ACT queue: the engines pipeline across the 8 span tiles.
    for t in range(NGATHER):
        g_t = gts[t]
        for k in range(2):
            j = 2 * t + k
            pc = psum_c.tile([128, D], F32, tag="pc")
            nc.tensor.matmul(out=pc[:], lhsT=asel[:, 128 * j:128 * (j + 1)],
                             rhs=t17[:], start=True, stop=True)
            d_t = dpool.tile([128, D], F32, tag="d")
            tt = nc.gpsimd.tensor_tensor(out=d_t[:], in0=g_t[:, k, :],
                                         in1=g_t[:, 2 + k, :],
                                         op=mybir.AluOpType.subtract)
            tt._wait_ge(gsems[t], 16)
            add_dep_helper(tt.ins, trigs[t].ins, sync=False,
                           reason="consume after trigger")
            m_t = dpool.tile([128, D], F32, tag="m")
            nc.scalar.mul(m_t[:], d_t[:], scale[:, j:j + 1])
            o_t = opool.tile([128, D], F32, tag="o")
            nc.vector.tensor_tensor(out=o_t[:], in0=m_t[:], in1=pc[:],
                                    op=mybir.AluOpType.add)
            nc.scalar.dma_start(out[128 * j:128 * (j + 1), :], o_t[:])


def build_nc():
    nc = bacc.Bacc("TRN2", target_bir_lowering=False, debug=False,
                   dynamic_dma_scratch_size=2 ** 16, num_swdge_queues=4)
    seq = nc.dram_tensor("seq", [S, D], F32, kind="ExternalInput")
    idx16 = nc.dram_tensor("idx16", [128, 128], I16, kind="ExternalInput")
    scale = nc.dram_tensor("scale", [128, NTILE], F32, kind="ExternalInput")
    asel = nc.dram_tensor("asel", [17, N], F16, kind="ExternalInput")
    utri = nc.dram_tensor("utri", [128, 128], F16, kind="ExternalInput")
    out = nc.dram_tensor("out", [N, D], F32, kind="ExternalOutput")
    from contextlib import ExitStack
    with tile.TileContext(nc) as tc:
        with ExitStack() as ctx:
            build_kernel_body(tc, seq.ap(), idx16.ap(), scale.ap(), asel.ap(),
                              utri.ap(), out.ap(), ctx)
    nc.compile()
    return nc


def host_precompute(span_indices: np.ndarray, span_indices_mask: np.ndarray):
    """Index-only preprocessing: gather idx list, per-span scale, offset
    selectors, triangular constant. Returns per-batch device input dicts."""
    spans = np.asarray(span_indices).astype(np.int64)      # [B, N, 2]
    mask = np.asarray(span_indices_mask).astype(np.int64)  # [B, N]
    starts = spans[..., 0]
    ends = spans[..., 1]
    widths = ends - starts                                  # >= 1

    # Gather token ids: token i holds sum seq[128*(i//128)..i]; E[e] = token
    # e-1, E[s] = token s-1, with s == 0 redirected to token 0 and
    # compensated via asel row 16 (+ token-0 value = seq row 0).
    tok_end = (ends - 1).astype(np.int64)                   # [B, N] in [0, S)
    tok_start = np.maximum(starts - 1, 0).astype(np.int64)

    # idx16[p, 32t + c] = list_t[c*16 + p%16];
    # list_t = [ends of spans 256t..256t+256) ++ starts of same]
    idx16 = np.empty((B, 128, 128), dtype=np.int16)
    for t in range(4):
        sl = slice(256 * t, 256 * t + 256)
        lst = np.concatenate([tok_end[:, sl], tok_start[:, sl]], axis=1)  # [B,512]
        wrapped = lst.reshape(B, 32, 16)                    # [B, c, p%16]
        block = np.transpose(wrapped, (0, 2, 1))            # [B, 16, 32]
        idx16[:, :, 32 * t:32 * t + 32] = np.tile(block, (1, 8, 1))

    # scale[p, j] = mask_n / width_n for n = 128j + p
    scale = (mask.astype(np.float32) /
             widths.astype(np.float32)).reshape(B, NTILE, 128)
    scale = np.ascontiguousarray(np.transpose(scale, (0, 2, 1)))  # [B,128,8]

    # asel[k, n]: correction selector.  C_n = sum_k asel[k, n] * T17[k]
    #   k < 16:  [k < blkE] - [k < blkS]   (blkS term dropped when start==0)
    #   k == 16: [start == 0]              (adds token-0 value = seq row 0)
    # out = (d + C_raw) * s == d*s + C with the per-span scale s folded in
    # here, so the device scales d on ACT and adds the PSUM correction on DVE.
    blk_e = tok_end // 128                                  # [B, N]
    blk_s = tok_start // 128
    ks = np.arange(16).reshape(1, 16, 1)
    a_e = (ks < blk_e[:, None, :])
    a_s = (ks < blk_s[:, None, :]) & (starts[:, None, :] > 0)
    s_n = (mask.astype(np.float32) / widths.astype(np.float32))[:, None, :]
    asel = np.zeros((B, 17, N), dtype=np.float32)
    asel[:, :16, :] = a_e.astype(np.float32) - a_s.astype(np.float32)
    asel[:, 16, :] = (starts == 0).astype(np.float32)
    asel = (asel * s_n).astype(np.float16)

    utri = np.triu(np.ones((128, 128), dtype=np.float16))

    return [{"idx16": np.ascontiguousarray(idx16[b]),
             "scale": np.ascontiguousarray(scale[b]),
             "asel": np.ascontiguousarray(asel[b]),
             "utri": utri} for b in range(B)]


def make_in_maps(sequence_tensor, span_indices, span_indices_mask):
    seq_f32 = np.ascontiguousarray(sequence_tensor, dtype=np.float32)
    host = host_precompute(span_indices, span_indices_mask)
    return [{"seq": seq_f32[b], **host[b]} for b in range(B)]


_NC_CACHE = None


def kernel(sequence_tensor: np.ndarray, span_indices: np.ndarray,
           span_indices_mask: np.ndarray) -> np.ndarray:
    global _NC_CACHE
    from concourse.bass_utils import run_bass_kernel_spmd

    if _NC_CACHE is None:
        _NC_CACHE = build_nc()
    nc = _NC_CACHE

    in_maps = make_in_maps(sequence_tensor, span_indices, span_indices_mask)
    res = run_bass_kernel_spmd(nc, in_maps, core_ids=list(range(B)))
    return np.stack([r["out"] for r in res.results], axis=0)


# revision 8
# speedup vs baseline: 1.3186x; 1.3186x over previous
"""AverageSpanExtractor Trainium2 kernel.

Math: out[b, n, :] = mean(seq[b, start_n:end_n, :]) * mask[b, n]

Strategy (per core; data-parallel over batch across 8 cores):
  1. Load seq [S=2048, D=512] f32, cast fp16.
  2. Per 128-token block: in-block inclusive cumsum via PE matmul with an
     upper-triangular ones matrix; cast PSUM to fp16 (ACT/DVE alternating)
     and store the UNOFFSET cumsums to a DRAM table [2048, 512] fp16
     (token i = in-block sum ending at i; no block offsets on this path).
  3. Gather token end-1 and token max(start-1, 0) for all spans with
     gpsimd.dma_gather on 4 parallel SWDGE queues (fp16 rows, 1KiB/desc).
  4. Post-gather correction: the missing block offsets (and the start==0
     edge case) are a tiny matmul C_j = A_j.T @ T17, where A_j [17, 128]
     is a host-computed selector (block-offset indicator * span scale) and
     T17 holds the 16 block totals (table row 127 of each block) plus
     token 0 (= seq row 0).  out_j = (G_end - G_start)*scale + C_j with the
     sub on GpSimd, scale-mul on ACT, add on DVE (parallel tail engines).
  5. All index-derived tensors (gather idx list, per-span scale, selectors)
     and the triangular constant come precomputed from the host.
"""

import numpy as np

import concourse.bacc as bacc
import concourse.bass as bass
import concourse.tile as tile
from concourse import mybir
from concourse.bass import AP
from concourse.library_config import mlp
from concourse.tile_rust import add_dep_helper

# Problem shape (hardcoded per contract).
B, S, D, N = 8, 2048, 512, 1024
NBLK = S // 128          # 16 token blocks
NTILE = N // 128         # 8 span tiles
NGATHER = 4              # gather instructions (2 span tiles each), 1 queue each
NQUAD = 4                # seq load / table store granularity: 4 blocks

F32 = mybir.dt.float32
I32 = mybir.dt.int32
I16 = mybir.dt.int16
F16 = mybir.dt.float16


def build_kernel_body(tc: tile.TileContext, seq: AP, idx16_in: AP, scale_in: AP,
                      asel_in: AP, utri_in: AP, out: AP, ctx):
    nc = tc.nc
    sbuf = ctx.enter_context(tc.tile_pool(name="sbuf", bufs=1))
    gpool = ctx.enter_context(tc.tile_pool(name="gpool", bufs=1))
    dpool = ctx.enter_context(tc.tile_pool(name="dpool", bufs=3))
    opool = ctx.enter_context(tc.tile_pool(name="opool", bufs=3))
    psum_e = ctx.enter_context(tc.tile_pool(name="pe", bufs=5, space="PSUM"))
    psum_c = ctx.enter_context(tc.tile_pool(name="pc", bufs=3, space="PSUM"))
    dram = ctx.enter_context(tc.tile_pool(name="dram", bufs=1, space="DRAM"))

    table = dram.tile([S, D], F16)

    # gather ucode load leads the GpSimd queue (~11us DMA trickle).
    nc.gpsimd.load_library(mlp)

    # ---------------- host-precomputed tensors (ACT queue) -----------------
    u_tri = sbuf.tile([128, 128], F16, tag="u_tri")
    nc.scalar.dma_start(u_tri[:], utri_in)
    idx16 = sbuf.tile([128, 128], I16, tag="idx16")
    nc.scalar.dma_start(idx16[:], idx16_in)
    scale = sbuf.tile([128, NTILE], F32, tag="scale")
    nc.scalar.dma_start(scale[:], scale_in)
    asel = sbuf.tile([17, N], F16, tag="asel")
    nc.scalar.dma_start(asel[:], asel_in)

    # ------- seq loads (pairs of blocks, alternating HWDGE queues), --------
    # ------- cast to fp16 on DVE as each pair lands ------------------------
    xbig = sbuf.tile([128, NBLK, D], F32, tag="xbig")
    xf = sbuf.tile([128, NBLK, D], F16, tag="xf")
    for h in range(NBLK // 2):
        sl = (slice(None), slice(2 * h, 2 * h + 2), slice(None))
        eng = nc.sync if h % 2 == 0 else nc.scalar
        eng.dma_start(
            xbig[sl],
            seq[256 * h:256 * (h + 1), :].rearrange("(j p) d -> p j d", p=128))
        nc.vector.tensor_copy(xf[sl], xbig[sl])

    # ------- prepare gathers early (idle Q7 cores), trigger later ----------
    # Traced BEFORE any table store so the preps carry no RAW dep on the
    # table; each trigger gets explicit deps on the stores instead.
    gsems = [ctx.enter_context(nc.semaphore(f"gsem{t}"))
             for t in range(NGATHER)]
    gts = []
    for t in range(NGATHER):
        g_t = gpool.tile([128, 4, D], F16, tag=f"g{t}")
        nc.gpsimd.dma_gather(
            out_ap=g_t[:], in_ap=table[:],
            idxs_ap=idx16[:, 32 * t:32 * t + 32],
            num_idxs=512, num_idxs_reg=512, elem_size=D,
            prepare_only=True, sem=gsems[t], queue_num=t)
        gts.append(g_t)

    # ---------------- in-block cumsums -> fp16 table stores ----------------
    # L_b = u_tri.T @ xf_b (inclusive cumsum); ACT casts PSUM f32 -> fp16
    # into ebig; one store DMA per quad of blocks on the Sync queue.
    ebig = sbuf.tile([128, NBLK, D], F16, tag="ebig")
    store_insts = []
    for q in range(NQUAD):
        for bb in range(4):
            b = 4 * q + bb
            pl = psum_e.tile([128, D], F32, tag="pe")
            nc.tensor.matmul(out=pl[:], lhsT=u_tri[:],
                             rhs=xf[:, b, :], start=True, stop=True)
            nc.scalar.copy(ebig[:, b, :], pl[:])
        store_insts.append(nc.sync.dma_start(
            table[512 * q:512 * (q + 1), :].rearrange("(j p) d -> p j d", p=128),
            ebig[:, 4 * q:4 * q + 4, :]))

    # T17: rows 0..15 = block totals (ebig partition 127), row 16 = token 0
    # (= seq row 0 = ebig[0, 0, :]) for the start==0 redirect.
    t17 = sbuf.tile([17, D], F16, tag="t17")
    nc.sync.dma_start(t17[0:NBLK, :], ebig[127:128, 0:NBLK, :])
    nc.sync.dma_start(t17[NBLK:NBLK + 1, :], ebig[0:1, 0, :])

    # ---------------- fire prepared gathers (4 parallel queues) ------------
    trigs = []
    for t in range(NGATHER):
        trig = nc.gpsimd.trigger_dma(count=None, queue_num=t)
        for st in store_insts:
            add_dep_helper(trig.ins, st.ins, sync=True,
                           reason="gather reads whole table")
        trigs.append(trig)

    # ---------------- combine: (G_end - G_start)*scale + A.T @ T17 ---------
    # sub on DVE (fp16 in), C-matmul on PE, scale-mul on ACT, add on DVE,
    # store on Sync. NB: no gpsimd elementwise ops anywhere — they would
    # force a ~10us ucode library reload after dma_gather.
    for t in range(NGATHER):
        g_t = gts[t]
        for k in range(2):
            j = 2 * t + k
            pc = psum_c.tile([128, D], F32, tag="pc")
            nc.tensor.matmul(out=pc[:], lhsT=asel[:, 128 * j:128 * (j + 1)],
                             rhs=t17[:], start=True, stop=True)
            d_t = dpool.tile([128, D], F32, tag="d")
            tt = nc.vector.tensor_tensor(out=d_t[:], in0=g_t[:, k, :],
                                         in1=g_t[:, 2 + k, :],
                                         op=mybir.AluOpType.subtract)
            tt._wait_ge(gsems[t], 16)
            add_dep_helper(tt.ins, trigs[t].ins, sync=False,
                           reason="consume after trigger")
            m_t = dpool.tile([128, D], F32, tag="m")
            nc.scalar.mul(m_t[:], d_t[:], scale[:, j:j + 1])
            o_t = opool.tile([128, D], F32, tag="o")
            nc.vector.tensor_tensor(out=o_t[:], in0=m_t[:], in1=pc[:],
                                    op=mybir.AluOpType.add)
            nc.sync.dma_start(out[128 * j:128 * (j + 1), :], o_t[:])


def build_nc():
    nc = bacc.Bacc("TRN2", target_bir_lowering=False, debug=False,
                   dynamic_dma_scratch_size=2 ** 16, num_swdge_queues=4)
    seq = nc.dram_tensor("seq", [S, D], F32, kind="ExternalInput")
    idx16 = nc.dram_tensor("idx16", [128, 128], I16, kind="ExternalInput")
    scale = nc.dram_tensor("scale", [128, NTILE], F32, kind="ExternalInput")
    asel = nc.dram_tensor("asel", [17, N], F16, kind="ExternalInput")
    utri = nc.dram_tensor("utri", [128, 128], F16, kind="ExternalInput")
    out = nc.dram_tensor("out", [N, D], F32, kind="ExternalOutput")
    from contextlib import ExitStack
    with tile.TileContext(nc) as tc:
        with ExitStack() as ctx:
            build_kernel_body(tc, seq.ap(), idx16.ap(), scale.ap(), asel.ap(),
                              utri.ap(), out.ap(), ctx)
    nc.compile()
    return nc


def host_precompute(span_indices: np.ndarray, span_indices_mask: np.ndarray):
    """Index-only preprocessing: gather idx list, per-span scale, offset
    selectors, triangular constant. Returns per-batch device input dicts."""
    spans = np.asarray(span_indices).astype(np.int64)      # [B, N, 2]
    mask = np.asarray(span_indices_mask).astype(np.int64)  # [B, N]
    starts = spans[..., 0]
    ends = spans[..., 1]
    widths = ends - starts                                  # >= 1

    # Gather token ids: token i holds sum seq[128*(i//128)..i]; E[e] = token
    # e-1, E[s] = token s-1, with s == 0 redirected to token 0 and
    # compensated via asel row 16 (+ token-0 value = seq row 0).
    tok_end = (ends - 1).astype(np.int64)                   # [B, N] in [0, S)
    tok_start = np.maximum(starts - 1, 0).astype(np.int64)

    # idx16[p, 32t + c] = list_t[c*16 + p%16];
    # list_t = [ends of spans 256t..256t+256) ++ starts of same]
    idx16 = np.empty((B, 128, 128), dtype=np.int16)
    for t in range(4):
        sl = slice(256 * t, 256 * t + 256)
        lst = np.concatenate([tok_end[:, sl], tok_start[:, sl]], axis=1)  # [B,512]
        wrapped = lst.reshape(B, 32, 16)                    # [B, c, p%16]
        block = np.transpose(wrapped, (0, 2, 1))            # [B, 16, 32]
        idx16[:, :, 32 * t:32 * t + 32] = np.tile(block, (1, 8, 1))

    # scale[p, j] = mask_n / width_n for n = 128j + p
    scale = (mask.astype(np.float32) /
             widths.astype(np.float32)).reshape(B, NTILE, 128)
    scale = np.ascontiguousarray(np.transpose(scale, (0, 2, 1)))  # [B,128,8]

    # asel[k, n]: correction selector.  C_n = sum_k asel[k, n] * T17[k]
    #   k < 16:  [k < blkE] - [k < blkS]   (blkS term dropped when start==0)
    #   k == 16: [start == 0]              (adds token-0 value = seq row 0)
    # out = (d + C_raw) * s == d*s + C with the per-span scale s folded in
    # here, so the device scales d on ACT and adds the PSUM correction on DVE.
    blk_e = tok_end // 128                                  # [B, N]
    blk_s = tok_start // 128
    ks = np.arange(16).reshape(1, 16, 1)
    a_e = (ks < blk_e[:, None, :])
    a_s = (ks < blk_s[:, None, :]) & (starts[:, None, :] > 0)
    s_n = (mask.astype(np.float32) / widths.astype(np.float32))[:, None, :]
    asel = np.zeros((B, 17, N), dtype=np.float32)
    asel[:, :16, :] = a_e.astype(np.float32) - a_s.astype(np.float32)
    asel[:, 16, :] = (starts == 0).astype(np.float32)
    asel = (asel * s_n).astype(np.float16)

    utri = np.triu(np.ones((128, 128), dtype=np.float16))

    return [{"idx16": np.ascontiguousarray(idx16[b]),
             "scale": np.ascontiguousarray(scale[b]),
             "asel": np.ascontiguousarray(asel[b]),
             "utri": utri} for b in range(B)]


def make_in_maps(sequence_tensor, span_indices, span_indices_mask):
    seq_f32 = np.ascontiguousarray(sequence_tensor, dtype=np.float32)
    host = host_precompute(span_indices, span_indices_mask)
    return [{"seq": seq_f32[b], **host[b]} for b in range(B)]


_NC_CACHE = None


def kernel(sequence_tensor: np.ndarray, span_indices: np.ndarray,
           span_indices_mask: np.ndarray) -> np.ndarray:
    global _NC_CACHE
    from concourse.bass_utils import run_bass_kernel_spmd

    if _NC_CACHE is None:
        _NC_CACHE = build_nc()
    nc = _NC_CACHE

    in_maps = make_in_maps(sequence_tensor, span_indices, span_indices_mask)
    res = run_bass_kernel_spmd(nc, in_maps, core_ids=list(range(B)))
    return np.stack([r["out"] for r in res.results], axis=0)
